# revision 18
# baseline (speedup 1.0000x reference)
"""Bass/Tile TRN2 kernel for a batched self-attention layer.

Reference computation (per batch b, N = 64*64 = 4096 tokens, C = 256, Dp = 32):
    f = input_h @ f_w          [N, Dp]
    g = x @ g_w                [N, Dp]
    s = g @ f.T                [N, N]
    beta = softmax(s, -1)
    o = beta @ input_h         [N, C]
    out = concat([o, x], -1)   [N, 2C]

Sharding: 8 cores = (batch b, query-half) pairs. Each core handles 2048 query
rows of one batch with the full 4096-key attention for that batch.

Per-core kernel design (v2 -- all matmul operands 16-bit, fp32 accumulation):
  * All input-layout work is done OFF the PE: f32 staging DMAs land in SBUF,
    DVE casts produce fp16 (projection path) and bf16 (PV path) copies, and
    the [token, channel] -> [channel, token] transposes for the f/g
    projections run on the DMA engines' XBAR transpose (16-bit SBUF->SBUF,
    one instruction per 128x512 slab) instead of PE transpose+copy round
    trips through PSUM.
  * Attention runs in TRANSPOSED layout: sT[key, query] = fT_chunk.T @ gT.
    fT is stored with chunk PAIRS interleaved across PE row groups and gT
    replicated 2x, so each QK pair issues 2 concurrent K=32 matmuls via
    tile_position row tiling.  (4-way row tiling would need 4 distinct PSUM
    banks per pair-step x2 ping-pong -- concurrent row-tiled matmuls that
    share a PSUM bank deadlock the PE -- which doesn't fit in 8 banks next
    to the output accumulators.)
    exp(sT) (Act engine, bf16 out, fp32 range so no max-subtraction) is
    directly the stationary operand of the PV matmul.
  * PV accumulates exp_chunk.T @ hR_chunk over the 32 key chunks into fp32
    PSUM; an appended ones-column makes column C the softmax denominator.
    Normalize with DVE reciprocal + tensor_scalar multiply, DMA out.
  * Scheduling: a burst of dummy warm-up matmuls runs during the framework
    preamble so the PE HAM clock gate is already at 2.4 GHz when real work
    arrives.  The h-block load loop runs query-block 0's QK->exp->PV
    pipeline one step behind the h stream, so the Act engine -- the
    steady-state pacer at ~1.15us per 1024-elem/lane exp -- fills while the
    6MB input stream is in flight.  qb1..3 run the same pipelined loop
    back-to-back; block-boundary norms hide under the next block's first
    exp.

The host-side wrapper shards inputs, runs the SPMD kernel on 8 cores, and
re-assembles the full [4, 64, 64, 512] output (the concat with x is pure data
movement, done on the host).
"""

import numpy as np

import concourse.bass as bass
import concourse.tile as tile
from concourse import bacc
from concourse import mybir
from concourse.bass_utils import run_bass_kernel_spmd

F32 = mybir.dt.float32
F16 = mybir.dt.float16
BF16 = mybir.dt.bfloat16

B, W, C, D = 4, 64, 256, 32
N = W * W                 # 4096 tokens (keys) per batch
NCORES = 8
SHARDS_PER_BATCH = NCORES // B   # 2
NQ = N // SHARDS_PER_BATCH       # 2048 query rows per core
KC = 128                         # key chunk (PE partition dim)
NKC = N // KC                    # 32 key chunks
NP = NKC // 2                    # 16 chunk pairs
NBLK = 8                         # h blocks of 512 keys (4 chunks each)
QBLK = 512                       # query block (moving free dim)
NQB = NQ // QBLK                 # 4 query blocks per core
NQSUB = QBLK // 128              # 4 query sub-tiles (PV stationary M)
Exp = mybir.ActivationFunctionType.Exp


def _build() -> bass.Bass:
    nc = bacc.Bacc("TRN2", target_bir_lowering=False)

    xs = nc.declare_dram_parameter("xs", [NQ, C], F32, isOutput=False)
    h = nc.declare_dram_parameter("h", [N, C], F32, isOutput=False)
    fw = nc.declare_dram_parameter("fw", [C, D], F32, isOutput=False)
    gw = nc.declare_dram_parameter("gw", [C, D], F32, isOutput=False)
    o = nc.declare_dram_parameter("o", [NQ, C], F32, isOutput=True)

    with tile.TileContext(nc) as tc:
        with (
            tc.tile_pool(name="const", bufs=1) as const_pool,
            tc.tile_pool(name="hr", bufs=1) as hr_pool,
            tc.tile_pool(name="hstage", bufs=8) as hstage_pool,
            tc.tile_pool(name="xstage", bufs=4) as xstage_pool,
            tc.tile_pool(name="c16", bufs=2) as c16_pool,
            tc.tile_pool(name="x16p", bufs=2) as x16_pool,
            tc.tile_pool(name="tsp", bufs=2) as tsp_pool,
            tc.tile_pool(name="proj", bufs=1) as proj_pool,
            tc.tile_pool(name="esb", bufs=3) as e_pool,
            tc.tile_pool(name="osb", bufs=4) as out_pool,
            tc.tile_pool(name="rsb", bufs=4) as r_pool,
        ):
            # ---- first thing: start the input DMA stream (x0, h0..h2) ----
            xst = {}
            hst = {}

            def issue_x_dma(b_):
                t = xstage_pool.tile([128, 4, C], F32, tag="xst", name=f"xst{b_}")
                nc.sync.dma_start(
                    out=t[:, :, :],
                    in_=xs[b_ * 512:(b_ + 1) * 512, :].rearrange(
                        "(j p) c -> p j c", p=128),
                )
                xst[b_] = t

            def issue_h_dma(p_):
                t = hstage_pool.tile([128, 4, C + 2], F32, tag="hst", name=f"hst{p_}")
                nc.sync.dma_start(
                    out=t[:, :, 0:C],
                    in_=h[p_ * 512:(p_ + 1) * 512, :].rearrange(
                        "(j p) c -> p j c", p=128),
                )
                hst[p_] = t

            # first sync-queue work: x0/h0/h1 input streams + the (tiny,
            # merged) weight loads -- everything the pipeline head needs
            issue_x_dma(0)
            fwg_st = const_pool.tile([128, 2, 2 * D], F32)
            nc.sync.dma_start(
                out=fwg_st[:, :, 0:D],
                in_=fw.rearrange("(cc p) d -> p cc d", p=128),
            )
            nc.sync.dma_start(
                out=fwg_st[:, :, D:2 * D],
                in_=gw.rearrange("(cc p) d -> p cc d", p=128),
            )
            issue_h_dma(0)
            issue_h_dma(1)
            issue_h_dma(2)

            zbias = const_pool.tile([128, 1], F32)
            nc.vector.memset(zbias[:, :], 0.0)
            fwg16 = const_pool.tile([128, 2, 2 * D], F16)

            # ---- PE warm-up: ~3.5us of junk matmuls so the HAM clock gate
            # flips to 2.4 GHz before the real pipeline starts ----
            warm_sb = const_pool.tile([128, 256], F16)
            nc.vector.memset(warm_sb[:, :], 0.0)
            nc.vector.tensor_copy(fwg16[:, :, :], fwg_st[:, :, :])
            with tc.tile_pool(name="warm", bufs=1, space="PSUM") as warm_pool:
                warm_ps = warm_pool.tile([128, 256], F32)
                for _ in range(16):
                    nc.tensor.matmul(
                        warm_ps[:, :], warm_sb[:, 0:128], warm_sb[:, :],
                        start=True, stop=True,
                    )

            # hR blocks: [128 keys, 4 chunks, C+2] bf16 (ones col at C), PV rhs.
            hr_blk = [
                hr_pool.tile([128, 4, C + 2], BF16, tag=f"hr{p}", name=f"hr{p}")
                for p in range(NBLK)
            ]
            # fT with chunk PAIRS interleaved across PE row groups
            # (rows 32i hold chunk 2g2+i).
            fT2 = proj_pool.tile([64, NP, 128], F16)
            # gT replicated across the 2 row groups.
            gT2 = proj_pool.tile([64, NQB, QBLK], F16)

            with (
                tc.tile_pool(name="sps", bufs=2, space="PSUM") as s_pool,
                tc.tile_pool(name="ops", bufs=1, space="PSUM") as o_pool,
            ):
                # ---------------- building blocks ----------------
                def process_x_block(b_):
                    """x staging -> fp16 (cc-major) -> DMA transpose -> gT2."""
                    x16 = x16_pool.tile([128, 2, 4, 128], F16, tag="x16", name=f"x16_{b_}")
                    for cc in range(2):
                        nc.vector.tensor_copy(
                            x16[:, cc, :, :],
                            xst[b_][:, :, cc * 128:(cc + 1) * 128],
                        )
                    xT = tsp_pool.tile([128, 2, 4, 128], F16, tag="xT", name=f"xT{b_}")
                    nc.sync.dma_start(
                        out=xT[:, :, :, :], in_=x16[:, :, :, :], transpose=True,
                    )
                    g_tile = s_pool.tile([128, 2, QBLK], F32, tag="s", name=f"gps{b_}")
                    g_ps = g_tile[0:2 * D, 0, :]
                    for i in range(2):
                        for cc in range(2):
                            nc.tensor.matmul(
                                g_ps[32 * i:32 * (i + 1), :],
                                fwg16[:, cc, D:2 * D],
                                xT[:, cc, :, :],
                                start=(cc == 0),
                                stop=(cc == 1),
                                tile_position=(0, 32 * i),
                            )
                    nc.vector.tensor_copy(gT2[:, b_, :], g_ps[:, :])

                def process_h_block(p_):
                    """h staging -> fp16 -> DMA transpose -> fT2 (+ hr bf16)."""
                    t = hst[p_]
                    # fp16 casts first: they gate the transpose -> f-proj chain
                    h16 = c16_pool.tile([128, 2, 4, 128], F16, tag="h16", name=f"h16_{p_}")
                    for cc in range(2):
                        nc.vector.tensor_copy(
                            h16[:, cc, :, :],
                            t[:, :, cc * 128:(cc + 1) * 128],
                        )
                    hT = tsp_pool.tile([128, 2, 4, 128], F16, tag="hT", name=f"hT{p_}")
                    nc.sync.dma_start(
                        out=hT[:, :, :, :], in_=h16[:, :, :, :], transpose=True,
                    )
                    nc.vector.memset(t[:, :, C:C + 1], 1.0)
                    nc.vector.memset(t[:, :, C + 1:C + 2], 0.0)
                    nc.vector.tensor_copy(hr_blk[p_][:, :, :], t[:, :, :])
                    f_tile = s_pool.tile([128, 2, QBLK], F32, tag="s", name=f"fps{p_}")
                    for j2 in range(2):
                        f_ps = f_tile[0:2 * D, j2, 0:128]
                        for i in range(2):
                            for cc in range(2):
                                nc.tensor.matmul(
                                    f_ps[32 * i:32 * (i + 1), :],
                                    fwg16[:, cc, 0:D],
                                    hT[:, cc, 2 * j2 + i, :],
                                    start=(cc == 0),
                                    stop=(cc == 1),
                                    tile_position=(0, 32 * i),
                                )
                    nc.vector.tensor_copy(
                        fT2[:, 2 * p_:2 * p_ + 2, :],
                        f_tile[0:2 * D, 0:2, 0:128],
                    )

                o_tiles = {}

                def get_o_tiles(qb):
                    o_tiles[qb] = [
                        o_pool.tile([128, C + 2], F32, tag=f"o{i}", name=f"ops{qb}_{i}")
                        for i in range(NQSUB)
                    ]
                    return o_tiles[qb]

                def qk_pair(qb, g2):
                    s_ps = s_pool.tile([128, 2, QBLK], F32, tag="s", name=f"s{qb}_{g2}")
                    for half in range(2):
                        nc.tensor.matmul(
                            s_ps[:, half, :],
                            fT2[32 * half:32 * (half + 1), g2, :],
                            gT2[32 * half:32 * (half + 1), qb, :],
                            start=True,
                            stop=True,
                            tile_position=(32 * half, 0),
                        )
                    return (qb, g2, s_ps)

                def exp_pv(item):
                    qb, g2, s_ps = item
                    e_sb = e_pool.tile([128, 2, QBLK], BF16, tag="e", name=f"e{qb}_{g2}")
                    nc.scalar.activation(e_sb[:, :, :], s_ps[:, :, :], Exp, bias=zbias[:, :])
                    o_ps = o_tiles[qb]
                    for half in range(2):
                        k = 2 * g2 + half
                        for i in range(NQSUB):
                            nc.tensor.matmul(
                                o_ps[i][:, :],
                                e_sb[:, half, 128 * i:128 * (i + 1)],
                                hr_blk[k // 4][:, k % 4, :],
                                start=(k == 0),
                                stop=(k == NKC - 1),
                            )
                    if g2 == NP - 1:
                        norm_out(qb)

                def norm_out(qb):
                    o_ps = o_tiles.pop(qb)
                    for i in range(NQSUB):
                        rec = r_pool.tile([128, 1], F32, tag="rec", name=f"rec{qb}_{i}")
                        nc.vector.reciprocal(rec[:, :], o_ps[i][:, C:C + 1])
                        out_sb = out_pool.tile([128, C], F32, tag="ob", name=f"ob{qb}_{i}")
                        nc.vector.tensor_scalar_mul(out_sb[:, :], o_ps[i][:, 0:C], rec[:, :])
                        r0 = qb * QBLK + i * 128
                        nc.sync.dma_start(out=o[r0:r0 + 128, :], in_=out_sb[:, :])

                # ---------------- load phase: h stream + qb0 attention ----
                process_x_block(0)
                get_o_tiles(0)
                prev = None
                for p in range(NBLK):
                    process_h_block(p)
                    # arrival-paced issue: keep ~3 h blocks in flight
                    if p + 3 < NBLK:
                        issue_h_dma(p + 3)
                    if p in (1, 2, 3):
                        issue_x_dma(p)           # x1 at p=1, x2 at p=2, x3 at p=3
                    if p in (2, 3, 4):
                        process_x_block(p - 1)   # x1 at p=2, x2 at p=3, x3 at p=4
                    for j2 in range(2):
                        cur = qk_pair(0, 2 * p + j2)
                        if prev is not None:
                            exp_pv(prev)
                        prev = cur

                # ---------------- steady state: qb1..qb3 ----------------
                for qb in range(1, NQB):
                    get_o_tiles(qb)
                    for g2 in range(NP):
                        cur = qk_pair(qb, g2)
                        exp_pv(prev)
                        prev = cur
                exp_pv(prev)

    nc.finalize()
    return nc


_CACHE: dict = {}


def _get_nc() -> bass.Bass:
    if "nc" not in _CACHE:
        _CACHE["nc"] = _build()
    return _CACHE["nc"]


def _shard(x, input_h, f_w, g_w):
    xf = np.ascontiguousarray(np.asarray(x, dtype=np.float32).reshape(B, N, C))
    hf = np.ascontiguousarray(np.asarray(input_h, dtype=np.float32).reshape(B, N, C))
    fwf = np.ascontiguousarray(np.asarray(f_w, dtype=np.float32).reshape(C, D))
    gwf = np.ascontiguousarray(np.asarray(g_w, dtype=np.float32).reshape(C, D))
    in_maps = []
    for c in range(NCORES):
        b, half = divmod(c, SHARDS_PER_BATCH)
        in_maps.append(
            {
                "xs": np.ascontiguousarray(xf[b, half * NQ:(half + 1) * NQ]),
                "h": hf[b],
                "fw": fwf,
                "gw": gwf,
            }
        )
    return in_maps


def _gather(results, x):
    of = np.empty((B, N, C), np.float32)
    for c in range(NCORES):
        b, half = divmod(c, SHARDS_PER_BATCH)
        of[b, half * NQ:(half + 1) * NQ] = results[c]["o"]
    o4 = of.reshape(B, W, W, C)
    x4 = np.asarray(x, dtype=np.float32).reshape(B, W, W, C)
    return np.concatenate([o4, x4], axis=-1)


def run(inputs: dict, trace: bool = False):
    """Run the kernel; returns (full_output, BassKernelResults)."""
    in_maps = _shard(**inputs)
    res = run_bass_kernel_spmd(_get_nc(), in_maps, list(range(NCORES)), trace=trace)
    out = _gather(res.results, inputs["x"])
    return out, res


def kernel(**inputs) -> np.ndarray:
    out, _ = run(inputs, trace=False)
    return out
